# revision 13
# baseline (speedup 1.0000x reference)
"""Distributed matvec kernel for nn_CubicalModel_ISM.

Computes Xp = I @ p, Yp = J @ p with I, J: [784, 50000], p: [50000], then
gathers tiny [50, 2] persistence diagrams from the 28x28 reshapes.

Strategy (8 NeuronCores):
  - Shard the contraction dim P=50000 column-wise across 8 cores
    (6400 = 50*128 per core, zero-padded at the tail).
  - Host-side: transpose each shard to [K, 784], split fp32 into bf16
    hi + bf16 lo planes (same total bytes as fp32, so the memory
    roofline is unchanged, but the PE runs at bf16 rate instead of the
    4x-slower fp32 mode), and pack everything into ONE interleaved
    DRAM tensor so each 256-row double-tile (all 4 planes: Ihi, Ilo,
    Jhi, Jlo) is a single fully-contiguous 3.2 MB DMA with 12.5 KB
    per-partition descriptors -- sequential HBM reads at line rate.
    p is split the same way; products hi*hi + hi*lo + lo*hi are
    accumulated in fp32 PSUM, recovering fp32-level precision
    (dropped lo*lo term is ~2^-18 relative).
  - Raw Bass (no Tile): this walrus build supports only ONE sync-wait
    per DMA instruction. Each DMA carries exactly one embedded wait --
    on its own round-robin lane's predecessor -- which strictly orders
    every lane's semaphore updates (race-free counts, 8 DMAs in
    flight). All other waits are standalone engine wait_ge ops.
  - Host: sum the 8 cores' partials (the "all-reduce"), reshape, gather.
"""

import numpy as np
import ml_dtypes

import concourse.bass as bass
import concourse.mybir as mybir
from concourse.bass_utils import run_bass_kernel_spmd

N_CORES = 8
P_FULL = 50000
H = W = 28
M = H * W  # 784
KT = 50  # k-subtiles (of 128) per core
K_PER = KT * 128  # 6400
NT = KT // 2  # 25 double-tiles
M2 = 2 * M  # 1568 bf16 cols per plane per double-tile
TILE_COLS = 4 * M2  # 6272 bf16 cols per double-tile (4 planes)
NHALF = 392  # 784 / 2, per-PSUM-bank output chunk

BF16 = ml_dtypes.bfloat16
F32 = np.float32

B = 8  # double-tile buffers (B x 12544B/partition SBUF)
N_LANES = 8


def build_nc() -> bass.Bass:
    f32 = mybir.dt.float32
    bf16 = mybir.dt.bfloat16
    nc = bass.Bass("TRN2")
    pw_d = nc.dram_tensor("pw", [128, 2 * KT], bf16, kind="ExternalInput")
    data_d = nc.dram_tensor(
        "data", [NT * 4 * 128, M2], bf16, kind="ExternalInput"
    )
    out_d = nc.dram_tensor("out", [6, M], f32, kind="ExternalOutput")

    # [NT, plane, 128, M2]: per (tile, plane) DMA = [128 x 3136B] contiguous,
    # consecutive DMAs walk HBM sequentially
    data_t = data_d[:, :].rearrange("(n f p) m -> n f p m", f=4, p=128)

    from contextlib import ExitStack

    with ExitStack() as stk:
        pw_sb = stk.enter_context(nc.sbuf_tensor("pw_sb", [128, 2 * KT], bf16))
        stream = stk.enter_context(
            nc.sbuf_tensor("stream", [128, B * TILE_COLS], bf16)
        )
        o_ih = stk.enter_context(nc.sbuf_tensor("o_ih", [2, M], f32))
        o_il = stk.enter_context(nc.sbuf_tensor("o_il", [1, M], f32))
        o_jh = stk.enter_context(nc.sbuf_tensor("o_jh", [2, M], f32))
        o_jl = stk.enter_context(nc.sbuf_tensor("o_jl", [1, M], f32))
        ps = {
            ("i", "h"): tuple(
                stk.enter_context(nc.psum_tensor(f"ps_ih{c}", [2, NHALF], f32))
                for c in range(2)
            ),
            ("i", "l"): tuple(
                stk.enter_context(nc.psum_tensor(f"ps_il{c}", [1, NHALF], f32))
                for c in range(2)
            ),
            ("j", "h"): tuple(
                stk.enter_context(nc.psum_tensor(f"ps_jh{c}", [2, NHALF], f32))
                for c in range(2)
            ),
            ("j", "l"): tuple(
                stk.enter_context(nc.psum_tensor(f"ps_jl{c}", [1, NHALF], f32))
                for c in range(2)
            ),
        }
        lanes = [
            stk.enter_context(nc.semaphore(f"dml{q}")) for q in range(N_LANES)
        ]
        pe_sem = stk.enter_context(nc.semaphore("pe_sem"))
        dve_sem = stk.enter_context(nc.semaphore("dve_sem"))
        act_sem = stk.enter_context(nc.semaphore("act_sem"))
        block = stk.enter_context(nc.Block())

        # plane order within a double-tile: ihi | ilo | jhi | jlo
        plane_off = {("i", "h"): 0, ("i", "l"): M2,
                     ("j", "h"): 2 * M2, ("j", "l"): 3 * M2}

        # Round-robin lane bookkeeping (see module docstring).
        lane_state = {"k": 0, "counts": [0] * N_LANES}
        dma_records = {}

        def issue_dma(sync, dst, src, record_key):
            q = lane_state["k"] % N_LANES
            lane_state["k"] += 1
            prev = lane_state["counts"][q]
            ins = sync.dma_start(dst, src).then_inc(lanes[q], 16)
            if prev > 0:
                ins.wait_op(lanes[q], 16 * prev, "sem-ge")
            lane_state["counts"][q] = prev + 1
            dma_records.setdefault(record_key, []).append((q, 16 * (prev + 1)))

        @block.sync
        def _(sync):
            issue_dma(sync, pw_sb[:, :], pw_d[:, :], ("pw",))
            for n in range(NT):
                if n >= B:
                    # slot n%B was last used by double-tile n-B; wait until
                    # the PE consumed it (pe_sem counts finished double-tiles)
                    sync.wait_ge(pe_sem, n - B + 1)
                s0 = (n % B) * TILE_COLS
                for f in range(4):
                    issue_dma(
                        sync,
                        stream[:, s0 + f * M2 : s0 + (f + 1) * M2],
                        data_t[n, f, :, :],
                        ("tile", n),
                    )
            # evict results once both eviction engines drained the PSUMs
            sync.wait_ge(dve_sem, 1)
            sync.wait_ge(act_sem, 1)
            issue_dma(sync, out_d[0:2, :], o_ih[:, :], ("out",))
            issue_dma(sync, out_d[2:3, :], o_il[:, :], ("out",))
            issue_dma(sync, out_d[3:5, :], o_jh[:, :], ("out",))
            issue_dma(sync, out_d[5:6, :], o_jl[:, :], ("out",))
            for q, v in dma_records[("out",)]:
                sync.wait_ge(lanes[q], v)

        @block.tensor
        def _(tensor):
            for n in range(NT):
                if n == 0:
                    for q, v in dma_records[("pw",)]:
                        tensor.wait_ge(lanes[q], v)
                for q, v in dma_records[("tile", n)]:
                    tensor.wait_ge(lanes[q], v)
                base = (n % B) * TILE_COLS
                last = None
                for two in range(2):
                    s = 2 * n + two  # k-subtile index
                    start = s == 0
                    stop = s == KT - 1
                    w2 = pw_sb[:, 2 * s : 2 * s + 2]  # [128, 2] (p_hi, p_lo)
                    w1 = pw_sb[:, 2 * s : 2 * s + 1]  # [128, 1] (p_hi)
                    for mat in ("i", "j"):
                        for c in range(2):
                            off_h = (base + plane_off[(mat, "h")]
                                     + two * M + c * NHALF)
                            off_l = (base + plane_off[(mat, "l")]
                                     + two * M + c * NHALF)
                            last = nc.tensor.matmul(
                                ps[(mat, "h")][c][:, :], w2,
                                stream[:, off_h : off_h + NHALF],
                                start=start, stop=stop,
                            )
                            last = nc.tensor.matmul(
                                ps[(mat, "l")][c][:, :], w1,
                                stream[:, off_l : off_l + NHALF],
                                start=start, stop=stop,
                            )
                last.then_inc(pe_sem, 1)

        @block.vector
        def _(vector):
            vector.wait_ge(pe_sem, NT)
            last = None
            for hl, dst in (("h", o_ih), ("l", o_il)):
                for c in range(2):
                    cs = slice(c * NHALF, (c + 1) * NHALF)
                    last = nc.vector.tensor_copy(
                        dst[:, cs], ps[("i", hl)][c][:, :]
                    )
            last.then_inc(dve_sem, 1)

        @block.scalar
        def _(scalar):
            scalar.wait_ge(pe_sem, NT)
            last = None
            for hl, dst in (("h", o_jh), ("l", o_jl)):
                for c in range(2):
                    cs = slice(c * NHALF, (c + 1) * NHALF)
                    last = nc.scalar.copy(dst[:, cs], ps[("j", hl)][c][:, :])
            last.then_inc(act_sem, 1)

    return nc


_NC_CACHE = None


def get_nc() -> bass.Bass:
    global _NC_CACHE
    if _NC_CACHE is None:
        _NC_CACHE = build_nc()
    return _NC_CACHE


def _split_hi_lo(a32: np.ndarray):
    hi = a32.astype(BF16)
    lo = (a32 - hi.astype(F32)).astype(BF16)
    return hi, lo


def shard_inputs(p, I, J) -> list[dict]:
    p = np.asarray(p, dtype=F32)
    I = np.asarray(I, dtype=F32)
    J = np.asarray(J, dtype=F32)

    p_pad = np.zeros(N_CORES * K_PER, dtype=F32)
    p_pad[:P_FULL] = p

    in_maps = []
    for c in range(N_CORES):
        lo_k = c * K_PER
        hi_k = min(lo_k + K_PER, P_FULL)
        kc = hi_k - lo_k

        pc = p_pad[c * K_PER : (c + 1) * K_PER]
        phi, plo = _split_hi_lo(pc)
        pw = np.zeros((128, 2 * KT), dtype=BF16)
        pw[:, 0::2] = phi.reshape(KT, 128).T
        pw[:, 1::2] = plo.reshape(KT, 128).T

        # data[n, f, p, two, m] = plane_f_kxm[n*256 + two*128 + p, m]
        data = np.zeros((NT, 4, 128, 2, M), dtype=BF16)
        for mi, mat in enumerate((I, J)):
            t = np.zeros((K_PER, M), dtype=F32)
            if kc > 0:
                t[:kc] = mat[:, lo_k:hi_k].T
            hi_p, lo_p = _split_hi_lo(t)
            for pi, plane in enumerate((hi_p, lo_p)):
                # [K_PER, M] -> [NT, two, 128, M] -> [NT, 128, two, M]
                v = plane.reshape(NT, 2, 128, M).transpose(0, 2, 1, 3)
                data[:, 2 * mi + pi, :, :, :] = v
        in_maps.append({
            "pw": pw,
            "data": data.reshape(NT * 4 * 128, M2),
        })
    return in_maps


def run(p, I, J, inds1, inds2, trace=False, **run_kwargs):
    """Returns ((dgm1, dgm2), BassKernelResults)."""
    in_maps = shard_inputs(p, I, J)
    nc = get_nc()
    res = run_bass_kernel_spmd(
        nc, in_maps, list(range(N_CORES)), trace=trace, **run_kwargs
    )
    acc = np.zeros((6, M), dtype=np.float64)
    for r in res.results:
        acc += r["out"].astype(np.float64)
    Xp = (acc[0] + acc[1] + acc[2]).astype(F32).reshape(H, W)
    Yp = (acc[3] + acc[4] + acc[5]).astype(F32).reshape(H, W)
    inds1 = np.asarray(inds1)
    inds2 = np.asarray(inds2)
    dgm1 = Xp[inds1[:, 0], inds1[:, 1]].reshape(-1, 2)
    dgm2 = Yp[inds2[:, 0], inds2[:, 1]].reshape(-1, 2)
    return (dgm1, dgm2), res


def kernel(p, I, J, inds1, inds2):
    out, _ = run(p, I, J, inds1, inds2, trace=False)
    return out
